# revision 1
# baseline (speedup 1.0000x reference)
"""Trainium2 Bass kernel for the DAMIC-style model:
embedding lookup -> 3x Conv1d(+ReLU+max-pool over tokens) -> BiLSTM over T -> sigmoid head.

Sharding: data-parallel over batch (B=32 -> 4 per core on 8 cores); weights
replicated; both LSTM directions computed per core on its own batch shard; the
host only reshapes/concats.

Pipeline: sentences are ordered (t, b) and conv runs in 8-timestep chunks
alternating from both ends of the sequence, so forward/reverse LSTM steps
interleave with conv on the PE as soon as their gate inputs are ready.
"""

import sys

sys.path.insert(0, "/opt/trn_rl_repo")

import numpy as np
import ml_dtypes

BF16 = ml_dtypes.bfloat16

VOCAB, EMB = 32000, 300
EMB_PAD = 384
NF = 100
FSIZES = (3, 4, 5)
NTAPS = 12
HID, OUT = 256, 32
B, T, L = 32, 64, 48
NCORES = 8
B_CORE = B // NCORES          # 4
S_CORE = B_CORE * T           # 256 sentences/core, ordered (t, b)
NTOK = S_CORE * L             # 12288
CH_T = 8                      # timesteps per conv chunk
NCH = T // CH_T               # 8 conv chunks
CHSENT = CH_T * B_CORE        # 32 sentences per chunk
CHTOK = CHSENT * L            # 1536 tokens per chunk
GCH = CHTOK // 128            # 12 gather chunks (128 tokens) per conv chunk
SENT_PER_PS = 8               # sentences per conv psum tile (= 2 timesteps)
PS_COLS = SENT_PER_PS * L     # 384
G4 = 4 * HID
CONV_ORDER = [0, 7, 1, 6, 2, 5, 3, 4]
PAIRS_OF = {3: [(0, 1)], 4: [(0, 1), (2, 3)], 5: [(0, 1), (2, 3)]}
SINGLES_OF = {3: [2], 4: [], 5: [4]}
PAIR_LIST = [(3, 0), (4, 0), (4, 2), (5, 0), (5, 2)]
PAIR_IDX = {p: i for i, p in enumerate(PAIR_LIST)}

_PROG = None


def build_program(debug=False):
    import concourse.bass as bass
    import concourse.tile as tile
    from concourse import bacc, mybir

    f32 = mybir.dt.float32
    bf16 = mybir.dt.bfloat16
    AF = mybir.ActivationFunctionType

    nc = bacc.Bacc("TRN2", target_bir_lowering=False, debug=False)

    idx_d = nc.dram_tensor("idx_w", [128, NTOK // 128], mybir.dt.int32, kind="ExternalInput").ap()
    iden_d = nc.dram_tensor("iden", [128, 128], bf16, kind="ExternalInput").ap()
    emb_d = nc.dram_tensor("emb_p", [VOCAB, EMB_PAD], bf16, kind="ExternalInput").ap()
    convw_d = nc.dram_tensor("convw", [128, 3, NTAPS * NF], bf16, kind="ExternalInput").ap()
    convw2_d = nc.dram_tensor("convw2", [128, 5 * NF], bf16, kind="ExternalInput").ap()
    convb_d = nc.dram_tensor("convb", [NF, 3], f32, kind="ExternalInput").ap()
    wih_d = nc.dram_tensor("wih", [NF, 3, 2, G4], bf16, kind="ExternalInput").ap()
    bih_d = nc.dram_tensor("bih", [2, 2, G4], bf16, kind="ExternalInput").ap()
    whh_d = nc.dram_tensor("whh", [128, 2, 2, 8, 128], bf16, kind="ExternalInput").ap()
    headw_d = nc.dram_tensor("headw", [128, 4, OUT], bf16, kind="ExternalInput").ap()
    headb_d = nc.dram_tensor("headb", [OUT, 1], f32, kind="ExternalInput").ap()
    out_d = nc.dram_tensor("out_t", [OUT, S_CORE], f32, kind="ExternalOutput").ap()
    if debug:
        dbg_feats_d = nc.dram_tensor("dbg_feats", [NF, 3, S_CORE], f32, kind="ExternalOutput").ap()
        dbg_xg_d = nc.dram_tensor("dbg_xg", [128, 2, T * 32], f32, kind="ExternalOutput").ap()
        dbg_h_d = nc.dram_tensor("dbg_h", [128, (T + 1) * 16], f32, kind="ExternalOutput").ap()

    tap_of = {3: 0, 4: 3, 5: 7}

    with tile.TileContext(nc) as tc:
        with (
            tc.tile_pool(name="const", bufs=1) as const,
            tc.tile_pool(name="gat", bufs=2) as gat,
            tc.tile_pool(name="gtok", bufs=4) as gtok,
            tc.tile_pool(name="small", bufs=3) as small,
            tc.tile_pool(name="cstate", bufs=2) as cstate,
            tc.tile_pool(name="cpsum", bufs=1, space="PSUM") as cpsum,
            tc.tile_pool(name="xpsum", bufs=2, space="PSUM") as xpsum,
            tc.tile_pool(name="gpsum", bufs=1, space="PSUM") as gpsum,
            tc.tile_pool(name="hpsum", bufs=1, space="PSUM") as hpsum,
        ):
            # gather-critical loads first (gpsimd queue feeds the gathers)
            idx_sb = const.tile([128, NTOK // 128], mybir.dt.int32)
            nc.gpsimd.dma_start(out=idx_sb[:], in_=idx_d[:])
            iden_sb = const.tile([128, 128], bf16)
            nc.gpsimd.dma_start(out=iden_sb[:], in_=iden_d[:])
            # weights on the HWDGE queue so they don't block gathers
            convw_sb = const.tile([128, 3, NTAPS * NF], bf16)
            nc.sync.dma_start(out=convw_sb[:], in_=convw_d[:])
            convw2_sb = const.tile([128, 5 * NF], bf16)
            nc.sync.dma_start(out=convw2_sb[:], in_=convw2_d[:])
            convb_sb = const.tile([NF, 3], f32)
            nc.sync.dma_start(out=convb_sb[:], in_=convb_d[:])
            wih_sb = const.tile([NF, 3, 2, G4], bf16)
            nc.sync.dma_start(out=wih_sb[:], in_=wih_d[:])
            bih_sb = const.tile([2, 2, G4], bf16)
            nc.sync.dma_start(out=bih_sb[:], in_=bih_d[:])
            whh_sb = const.tile([128, 2, 2, 8, 128], bf16)
            nc.sync.dma_start(out=whh_sb[:], in_=whh_d[:])
            headw_sb = const.tile([128, 4, OUT], bf16)
            nc.sync.dma_start(out=headw_sb[:], in_=headw_d[:])
            headb_sb = const.tile([OUT, 1], f32)
            nc.sync.dma_start(out=headb_sb[:], in_=headb_d[:])

            ones_sb = const.tile([2, S_CORE], bf16)
            nc.vector.memset(ones_sb[:], 1.0)

            feats = [const.tile([NF, S_CORE], bf16, tag=f"f{fs}", name=f"f{fs}") for fs in FSIZES]
            # xgT: [128, (d, t, g, b)] fp32 — both dirs in one tile
            xgT = const.tile([128, 2, T, 8, B_CORE], f32)
            # hseq: [128, slot, dir, ktile, b] bf16; slot 0 = h0 = 0
            hseq = const.tile([128, T + 1, 2, 2, B_CORE], bf16)
            nc.vector.memset(hseq[:, 0], 0.0)
            c_prev = []
            for d in range(2):
                c0 = cstate.tile([128, 2, B_CORE], f32, tag=f"c{d}", name=f"c0{d}")
                nc.vector.memset(c0[:], 0.0)
                c_prev.append(c0)

            def conv_chunk(sc):
                g = gat.tile([128, 3, CHTOK], bf16, tag="g", name="g")
                for c in range(GCH):
                    gc = GCH * sc + c
                    gt = gtok.tile([128, EMB_PAD], bf16, tag="gt", name="gt")
                    nc.gpsimd.indirect_dma_start(
                        out=gt[:], out_offset=None, in_=emb_d[:],
                        in_offset=bass.IndirectOffsetOnAxis(
                            ap=idx_sb[:, gc : gc + 1], axis=0
                        ),
                    )
                    for e in range(3):
                        nc.sync.dma_start_transpose(
                            g[:, e, 128 * c : 128 * (c + 1)],
                            gt[:, 128 * e : 128 * (e + 1)],
                        )
                # tap-tail pairing: rows 64:108 of block 2 = rows 0:44 shifted by 1 token
                nc.gpsimd.dma_start(
                    out=g[64:108, 2, 0 : CHTOK - 1], in_=g[0:44, 2, 1:CHTOK]
                )
                for j in range(CHSENT // SENT_PER_PS):  # 4 psum tiles
                    base = PS_COLS * j
                    s0 = CHSENT * sc + SENT_PER_PS * j
                    for fi, fs in enumerate(FSIZES):
                        ps = cpsum.tile([NF, PS_COLS], f32, tag=f"ps{fs}", name=f"ps{fs}")
                        mms = []
                        for kk in range(2):
                            for k in range(fs):
                                ti = tap_of[fs] + k
                                mms.append((convw_sb[:, kk, ti * NF : (ti + 1) * NF], kk, k, 0))
                        for (pa, pb) in PAIRS_OF[fs]:
                            pi = PAIR_IDX[(fs, pa)]
                            mms.append((convw2_sb[:, pi * NF : (pi + 1) * NF], 2, pa, 1))
                        for k in SINGLES_OF[fs]:
                            ti = tap_of[fs] + k
                            mms.append((convw_sb[:, 2, ti * NF : (ti + 1) * NF], 2, k, 0))
                        for mm, (lhsT, kk, k, is_pair) in enumerate(mms):
                            n = min(PS_COLS, CHTOK - is_pair - base - k)
                            nc.tensor.matmul(
                                ps[:, 0:n],
                                lhsT,
                                g[:, kk, base + k : base + k + n],
                                start=(mm == 0),
                                stop=(mm == len(mms) - 1),
                            )
                        ps3 = ps[:].rearrange("p (s l) -> p s l", l=L)
                        nc.vector.reduce_max(
                            out=feats[fi][:, s0 : s0 + SENT_PER_PS],
                            in_=ps3[:, :, 0 : L - fs + 1],
                            axis=mybir.AxisListType.X,
                        )
                for fi in range(3):
                    sl = slice(CHSENT * sc, CHSENT * (sc + 1))
                    nc.scalar.activation(
                        out=feats[fi][:, sl], in_=feats[fi][:, sl], func=AF.Relu,
                        bias=convb_sb[:, fi : fi + 1],
                    )

            def xg_chunk(sc):
                cols = slice(CHSENT * sc, CHSENT * (sc + 1))  # feats cols (t,b)
                for d in range(2):
                    for gt in range(8):
                        ps = xpsum.tile([128, CHSENT], f32, tag="xp", name="xp")
                        for kk in range(3):
                            nc.tensor.matmul(
                                ps[:],
                                wih_sb[:, kk, d, 128 * gt : 128 * (gt + 1)],
                                feats[kk][:, cols],
                                start=(kk == 0),
                                stop=False,
                            )
                        nc.tensor.matmul(
                            ps[:],
                            bih_sb[:, d, 128 * gt : 128 * (gt + 1)],
                            ones_sb[:, cols],
                            start=False,
                            stop=True,
                        )
                        # psum cols (t,b) -> xgT[:, d, t, gt, b]
                        nc.scalar.copy(
                            out=xgT[:, d, CH_T * sc : CH_T * (sc + 1), gt, :],
                            in_=ps[:].rearrange("p (t b) -> p t b", b=B_CORE),
                        )

            def lstm_step(s):
                # fwd(t=s) and rev(tt=T-1-s) emitted as two independent chains
                for d in range(2):
                    tt = s if d == 0 else T - 1 - s
                    rslot = (s if d == 0 else (T + 1 - s)) if s > 0 else 0
                    ps = gpsum.tile([128, 32], f32, tag=f"gp{d}", name=f"gp{d}")
                    for gt in range(8):
                        for kk in range(2):
                            nc.tensor.matmul(
                                ps[:, 4 * gt : 4 * gt + 4],
                                whh_sb[:, d, kk, gt, :],
                                hseq[:, rslot, d, kk, :],
                                start=(kk == 0),
                                stop=(kk == 1),
                            )
                    gates = small.tile([128, 32], f32, tag=f"gates{d}", name=f"gates{d}")
                    nc.vector.tensor_add(
                        gates[:],
                        ps[:],
                        xgT[:, d, tt, :, :].rearrange("p g b -> p (g b)"),
                    )
                    sig = small.tile([128, 24], f32, tag=f"sig{d}", name=f"sig{d}")
                    nc.scalar.activation(sig[:], gates[:, 0:24], AF.Sigmoid)
                    tg = small.tile([128, 8], f32, tag=f"tg{d}", name=f"tg{d}")
                    nc.scalar.activation(tg[:], gates[:, 24:32], AF.Tanh)
                    t1 = small.tile([128, 8], f32, tag=f"t1{d}", name=f"t1{d}")
                    nc.vector.tensor_mul(t1[:], sig[:, 0:8], tg[:])
                    cn = cstate.tile([128, 2, B_CORE], f32, tag=f"c{d}", name=f"c{d}")
                    nc.vector.tensor_mul(cn[:], sig[:, 8:16], c_prev[d][:])
                    nc.vector.tensor_add(cn[:], cn[:], t1[:])
                    c_prev[d] = cn
                    thc = small.tile([128, 8], f32, tag=f"thc{d}", name=f"thc{d}")
                    nc.scalar.activation(thc[:], cn[:], AF.Tanh)
                    nc.vector.tensor_mul(hseq[:, tt + 1, d], sig[:, 16:24], thc[:])

            done = set()
            state = {"emitted": 0}

            def ready_steps():
                while state["emitted"] < T:
                    s = state["emitted"]
                    if (s // CH_T) in done and ((T - 1 - s) // CH_T) in done:
                        lstm_step(s)
                        state["emitted"] += 1
                    else:
                        break

            for sc in CONV_ORDER:
                conv_chunk(sc)
                xg_chunk(sc)
                done.add(sc)
                ready_steps()
            assert state["emitted"] == T

            if debug:
                dbg_f = const.tile([NF, 3, S_CORE], f32)
                for fi in range(3):
                    nc.vector.tensor_copy(dbg_f[:, fi, :], feats[fi][:])
                nc.gpsimd.dma_start(out=dbg_feats_d[:], in_=dbg_f[:])
                xg_flat = xgT[:].rearrange("p d t g b -> p d (t g b)")
                nc.gpsimd.dma_start(out=dbg_xg_d[:], in_=xg_flat)
                hf32 = const.tile([128, (T + 1) * 16], f32)
                nc.vector.tensor_copy(hf32[:], hseq[:].rearrange("p a b c d -> p (a b c d)"))
                nc.gpsimd.dma_start(out=dbg_h_d[:], in_=hf32[:])

            # head: out.T[o, (b,t)] = sigmoid(head_w @ h2 + b)
            hp = hpsum.tile([OUT, S_CORE], f32)
            for qd in range(4):
                d, kk = qd // 2, qd % 2
                rhs = hseq[:, 1 : T + 1, d, kk, :].rearrange("p t b -> p b t")
                nc.tensor.matmul(
                    hp[:], headw_sb[:, qd, :], rhs, start=(qd == 0), stop=(qd == 3)
                )
            out_sb = small.tile([OUT, S_CORE], f32, tag="outsb", name="outsb")
            nc.scalar.activation(out_sb[:], hp[:], AF.Sigmoid, bias=headb_sb[:])
            nc.gpsimd.dma_start(out=out_d[:], in_=out_sb[:])

    nc.compile()
    return nc


def get_program():
    global _PROG
    if _PROG is None:
        _PROG = build_program()
    return _PROG


# ------------- host-side data prep (reshape/transpose/pad/cast only) -------------

def prep_shared(inputs):
    emb = np.zeros((VOCAB, EMB_PAD), np.float32)
    emb[:, :EMB] = inputs["emb"]
    emb_p = emb.astype(BF16)

    Wfull = np.zeros((EMB_PAD, NTAPS * NF), np.float32)
    col = 0
    for fs in FSIZES:
        w = np.asarray(inputs[f"conv_w{fs}"], np.float32)
        for k in range(fs):
            Wfull[:EMB, col : col + NF] = w[:, :, k].T
            col += NF
    convw = Wfull.reshape(3, 128, NTAPS * NF).transpose(1, 0, 2).astype(BF16)

    convb = np.stack(
        [np.asarray(inputs[f"conv_b{fs}"], np.float32) for fs in FSIZES], axis=1
    )

    convw2 = np.zeros((128, 5 * NF), np.float32)
    for i, (fs, ka) in enumerate(PAIR_LIST):
        w = np.asarray(inputs[f"conv_w{fs}"], np.float32)
        convw2[0:44, i * NF : (i + 1) * NF] = w[:, 256:300, ka].T
        convw2[64:108, i * NF : (i + 1) * NF] = w[:, 256:300, ka + 1].T

    perm = np.concatenate(
        [np.arange(0, 256), np.arange(256, 512), np.arange(768, 1024), np.arange(512, 768)]
    )  # i,f,g,o -> i,f,o,g

    wih_h = np.zeros((NF, 3, 2, G4), np.float32)
    bih_h = np.zeros((2, 2, G4), np.float32)
    whh_h = np.zeros((128, 2, 2, 8, 128), np.float32)
    for d, tag in ((0, "f"), (1, "r")):
        wih = np.asarray(inputs[f"w_ih_{tag}"], np.float32)[perm]
        whh = np.asarray(inputs[f"w_hh_{tag}"], np.float32)[perm]
        bih_h[0, d] = np.asarray(inputs[f"b_ih_{tag}"], np.float32)[perm]
        bih_h[1, d] = np.asarray(inputs[f"b_hh_{tag}"], np.float32)[perm]
        for kk in range(3):
            wih_h[:, kk, d, :] = wih[:, kk * NF : (kk + 1) * NF].T
        whh_h[:, d] = whh.reshape(8, 128, 2, 128).transpose(3, 2, 0, 1)

    headw = np.asarray(inputs["head_w"], np.float32)
    headw_h = headw.T.reshape(4, 128, OUT).transpose(1, 0, 2).astype(BF16)
    headb_h = np.asarray(inputs["head_b"], np.float32).reshape(OUT, 1)

    return {
        "emb_p": emb_p,
        "convw": np.ascontiguousarray(convw),
        "convw2": np.ascontiguousarray(convw2.astype(BF16)),
        "convb": np.ascontiguousarray(convb),
        "wih": wih_h.astype(BF16),
        "bih": bih_h.astype(BF16),
        "whh": np.ascontiguousarray(whh_h.astype(BF16)),
        "headw": np.ascontiguousarray(headw_h),
        "headb": headb_h,
        "iden": np.eye(128, dtype=BF16),
    }


def prep_core_idx(dialogue, core):
    """(t, b)-ordered token stream; token c*128+p at [p, c]."""
    dia = np.asarray(dialogue[B_CORE * core : B_CORE * (core + 1)], np.int32)
    ids = dia.transpose(1, 0, 2).reshape(-1)  # (t, b, l)
    return np.ascontiguousarray(ids.reshape(NTOK // 128, 128).T)


def kernel(**inputs):
    from concourse.bass_utils import run_bass_kernel_spmd

    nc = get_program()
    shared = prep_shared(inputs)
    dialogue = np.asarray(inputs["dialogue"])
    in_maps = []
    for core in range(NCORES):
        m = dict(shared)
        m["idx_w"] = prep_core_idx(dialogue, core)
        in_maps.append(m)
    res = run_bass_kernel_spmd(nc, in_maps, list(range(NCORES)))
    out = np.zeros((B, T, OUT), np.float32)
    for core in range(NCORES):
        o = res.results[core]["out_t"]  # [32, 256] col = b*64 + t
        out[B_CORE * core : B_CORE * (core + 1)] = o.reshape(OUT, B_CORE, T).transpose(
            1, 2, 0
        )
    return out

